# revision 1
# baseline (speedup 1.0000x reference)
"""Trainium2 Bass kernel for DeepLACForests (8-core data-parallel over batch).

Computation (matches the reference nn.Module):
  feats = relu(X @ W1 + b1)                  [B, 128]
  prediction = feats @ W2 + b2               [B, 10]
  dec = sigmoid(einsum('bd,tid->tib', feats, nodeW) + nodeb)   [16, 255, B]
  all_results = level-wise prefix product down a depth-9 heap  [16, 511, B]

Sharding: data-parallel over batch (1024 rows/core x 8 cores); encoder and
node weights (~2MB) replicated. On-device layout keeps batch on SBUF
partitions and (tree, node) on the free dimension so every DMA is fully
contiguous; the host pre-transposes X / nodeW and untransposes the result.
"""

from contextlib import ExitStack

import numpy as np

import concourse.bass as bass
import concourse.tile as tile
from concourse import bacc, mybir
from concourse.bass_utils import run_bass_kernel_spmd

F32 = mybir.dt.float32
AF = mybir.ActivationFunctionType

N_CORES = 8
BATCH = 8192
BC = BATCH // N_CORES          # 1024 batch rows per core
NB = BC // 128                 # 8 partition tiles per core
IN_DIM = 784
HID = 128
CLS = 10
ENSEMBLE = 16
INTERNAL = 255                 # internal nodes per tree
TOTAL = 511                    # all nodes per tree
DEPTH = 9
NODES = ENSEMBLE * INTERNAL    # 4080 decision columns
NCHUNK = 510                   # nodes per matmul (<=512 fp32 moving-operand max)
NCHUNKS = NODES // NCHUNK      # 8
K_SIZES = [128] * 6 + [16]     # 784 = 6*128 + 16 contraction chunks


def build_bass():
    nc = bacc.Bacc("TRN2", target_bir_lowering=False, debug=False,
                   num_devices=N_CORES)

    xt_d = nc.dram_tensor("xt", [IN_DIM, BC], F32, kind="ExternalInput")
    w1_d = nc.dram_tensor("w1", [IN_DIM, HID], F32, kind="ExternalInput")
    b1_d = nc.dram_tensor("b1", [HID, 1], F32, kind="ExternalInput")
    w2_d = nc.dram_tensor("w2", [HID, CLS], F32, kind="ExternalInput")
    b2_d = nc.dram_tensor("b2", [1, CLS], F32, kind="ExternalInput")
    nwt_d = nc.dram_tensor("nwt", [HID, NODES], F32, kind="ExternalInput")
    nb_d = nc.dram_tensor("nb", [1, NODES], F32, kind="ExternalInput")

    pred_d = nc.dram_tensor("pred", [BC, CLS], F32, kind="ExternalOutput")
    res_d = nc.dram_tensor("res", [NB, 128, ENSEMBLE, TOTAL], F32,
                           kind="ExternalOutput")

    with tile.TileContext(nc) as tc, ExitStack() as ctx:
        consts = ctx.enter_context(tc.tile_pool(name="consts", bufs=1))
        xtp = ctx.enter_context(tc.tile_pool(name="xtp", bufs=1))
        fp = ctx.enter_context(tc.tile_pool(name="fp", bufs=1))
        enc_ps = ctx.enter_context(
            tc.tile_pool(name="enc_ps", bufs=2, space="PSUM"))
        pred_ps = ctx.enter_context(
            tc.tile_pool(name="pred_ps", bufs=2, space="PSUM"))
        tree_ps = ctx.enter_context(
            tc.tile_pool(name="tree_ps", bufs=2, space="PSUM"))
        decp = ctx.enter_context(tc.tile_pool(name="decp", bufs=2))
        resp = ctx.enter_context(tc.tile_pool(name="resp", bufs=2))
        outp = ctx.enter_context(tc.tile_pool(name="outp", bufs=2))

        # ---- constant/weight loads ----
        nwt_sb = consts.tile([HID, NODES], F32, tag="nwt")
        nc.sync.dma_start(out=nwt_sb, in_=nwt_d[:])
        w1_sb = []
        off = 0
        for k, ksz in enumerate(K_SIZES):
            t = consts.tile([128, HID], F32, tag=f"w1_{k}")
            nc.sync.dma_start(out=t[:ksz], in_=w1_d[off:off + ksz, :])
            w1_sb.append(t)
            off += ksz
        b1_sb = consts.tile([HID, 1], F32, tag="b1")
        nc.sync.dma_start(out=b1_sb, in_=b1_d[:])
        w2_sb = consts.tile([HID, CLS], F32, tag="w2")
        nc.sync.dma_start(out=w2_sb, in_=w2_d[:])
        b2_sb = consts.tile([1, CLS], F32, tag="b2")
        nc.sync.dma_start(out=b2_sb, in_=b2_d[:])
        nb_sb = consts.tile([1, NODES], F32, tag="nb")
        nc.sync.dma_start(out=nb_sb, in_=nb_d[:])
        ones_sb = consts.tile([1, 128], F32, tag="ones")
        nc.vector.memset(ones_sb, 1.0)

        xt_sb = []
        off = 0
        for k, ksz in enumerate(K_SIZES):
            t = xtp.tile([128, BC], F32, tag=f"xt_{k}")
            nc.sync.dma_start(out=t[:ksz], in_=xt_d[off:off + ksz, :])
            xt_sb.append(t)
            off += ksz

        # ---- encoder: featsT[h, b] = relu(sum_d W1[d,h] * XT[d,b] + b1[h]) ----
        featsT = fp.tile([HID, BC], F32, tag="featsT")
        for n in range(BC // 512):
            ps = enc_ps.tile([128, 512], F32, tag="enc")
            for k, ksz in enumerate(K_SIZES):
                nc.tensor.matmul(ps, w1_sb[k][:ksz],
                                 xt_sb[k][:ksz, n * 512:(n + 1) * 512],
                                 start=(k == 0), stop=(k == len(K_SIZES) - 1))
            nc.scalar.activation(out=featsT[:, n * 512:(n + 1) * 512], in_=ps,
                                 func=AF.Relu, bias=b1_sb, scale=1.0)

        # ---- aux prediction: pred[b, c] = feats @ W2 + b2 ----
        for i in range(NB):
            bsl = featsT[:, i * 128:(i + 1) * 128]
            pp = pred_ps.tile([128, CLS], F32, tag="pred")
            nc.tensor.matmul(pp, bsl, w2_sb, start=True, stop=False)
            nc.tensor.matmul(pp, ones_sb, b2_sb, start=False, stop=True)
            po = outp.tile([128, CLS], F32, tag="po")
            nc.vector.tensor_copy(po, pp)
            nc.sync.dma_start(out=pred_d[i * 128:(i + 1) * 128, :], in_=po)

        # ---- trees, per 128-row batch tile ----
        for i in range(NB):
            bsl = featsT[:, i * 128:(i + 1) * 128]
            dec = decp.tile([128, NODES], F32, tag="dec")
            for half in range(NCHUNKS // 2):
                ps = tree_ps.tile([128, 2, 512], F32, tag="tree")
                for c2 in range(2):
                    c = half * 2 + c2
                    csl = slice(c * NCHUNK, (c + 1) * NCHUNK)
                    nc.tensor.matmul(ps[:, c2, :NCHUNK], bsl, nwt_sb[:, csl],
                                     start=True, stop=False)
                    nc.tensor.matmul(ps[:, c2, :NCHUNK], ones_sb, nb_sb[:, csl],
                                     start=False, stop=True)
                dsl = dec[:, half * 2 * NCHUNK:(half + 1) * 2 * NCHUNK]
                nc.scalar.activation(
                    out=dsl.rearrange("p (x i) -> p x i", x=2),
                    in_=ps[:, :, :NCHUNK], func=AF.Sigmoid)

            dec16 = dec.rearrange("p (t i) -> p t i", t=ENSEMBLE)
            res_sb = resp.tile([128, ENSEMBLE, TOTAL], F32, tag="res")
            nc.vector.memset(res_sb[:, :, 0:1], 1.0)
            for lvl in range(DEPTH - 1):
                start = (1 << lvl) - 1
                cnt = 1 << lvl
                parent = res_sb[:, :, start:start + cnt]
                decs = dec16[:, :, start:start + cnt]
                ch = res_sb[:, :, 2 * start + 1: 2 * start + 1 + 2 * cnt]
                ch = ch.rearrange("p t (k two) -> p t k two", two=2)
                left = ch[:, :, :, 0]
                right = ch[:, :, :, 1]
                nc.vector.tensor_mul(left, parent, decs)
                nc.vector.tensor_sub(right, parent, left)
            nc.sync.dma_start(out=res_d[i], in_=res_sb)

    nc.compile()
    return nc


_NC_CACHE = None


def _get_nc():
    global _NC_CACHE
    if _NC_CACHE is None:
        _NC_CACHE = build_bass()
    return _NC_CACHE


def run_on_device(inputs, trace=False):
    """Shard inputs, run the Bass kernel on 8 cores, gather full outputs.

    Returns (prediction, all_results, BassKernelResults)."""
    X = np.ascontiguousarray(np.asarray(inputs["X"], dtype=np.float32))
    W1 = np.ascontiguousarray(np.asarray(inputs["W1"], dtype=np.float32))
    b1 = np.ascontiguousarray(
        np.asarray(inputs["b1"], dtype=np.float32).reshape(HID, 1))
    W2 = np.ascontiguousarray(np.asarray(inputs["W2"], dtype=np.float32))
    b2 = np.ascontiguousarray(
        np.asarray(inputs["b2"], dtype=np.float32).reshape(1, CLS))
    nodeW = np.asarray(inputs["nodeW"], dtype=np.float32)
    nodeb = np.ascontiguousarray(
        np.asarray(inputs["nodeb"], dtype=np.float32).reshape(1, NODES))

    XT = np.ascontiguousarray(X.T)                                 # [784, 8192]
    nwt = np.ascontiguousarray(nodeW.reshape(NODES, HID).T)        # [128, 4080]

    in_maps = []
    for c in range(N_CORES):
        in_maps.append({
            "xt": np.ascontiguousarray(XT[:, c * BC:(c + 1) * BC]),
            "w1": W1, "b1": b1, "w2": W2, "b2": b2,
            "nwt": nwt, "nb": nodeb,
        })

    nc = _get_nc()
    r = run_bass_kernel_spmd(nc, in_maps, core_ids=list(range(N_CORES)),
                             trace=trace)

    prediction = np.empty((BATCH, CLS), np.float32)
    all_results = np.empty((ENSEMBLE, TOTAL, BATCH), np.float32)
    for c in range(N_CORES):
        prediction[c * BC:(c + 1) * BC] = r.results[c]["pred"]
        # res: [NB, 128, 16, 511] -> [16, 511, BC]
        rc = r.results[c]["res"].reshape(BC, ENSEMBLE, TOTAL)
        all_results[:, :, c * BC:(c + 1) * BC] = rc.transpose(1, 2, 0)
    return prediction, all_results, r


def kernel(**inputs):
    prediction, all_results, _ = run_on_device(inputs, trace=False)
    return prediction, all_results


# revision 3
# speedup vs baseline: 1.9446x; 1.9446x over previous
"""Trainium2 Bass kernel for DeepLACForests (8-core data-parallel over batch).

Computation (matches the reference nn.Module):
  feats = relu(X @ W1 + b1)                  [B, 128]
  prediction = feats @ W2 + b2               [B, 10]
  dec = sigmoid(einsum('bd,tid->tib', feats, nodeW) + nodeb)   [16, 255, B]
  all_results = level-wise prefix product down a depth-9 heap  [16, 511, B]

Sharding: data-parallel over batch (1024 rows/core x 8 cores); encoder and
node weights (~2MB) replicated. On-device layout keeps batch on SBUF
partitions and (tree, node) on the free dimension so every DMA is fully
contiguous; the host pre-transposes X / nodeW and untransposes the result.

Matmuls run as bf16 hi/lo split-precision (x = hi + lo exactly, with
x @ w = hi@whi + lo@whi + hi@wlo up to ~2^-17 relative error): fp32
matmuls stream at half rate on the PE, so three bf16 passes beat one
fp32 pass while keeping near-fp32 accuracy.
"""

from contextlib import ExitStack

import ml_dtypes
import numpy as np

import concourse.bass as bass
import concourse.tile as tile
from concourse import bacc, mybir
from concourse.bass_utils import run_bass_kernel_spmd

F32 = mybir.dt.float32
BF16 = mybir.dt.bfloat16
AF = mybir.ActivationFunctionType

N_CORES = 8
BATCH = 8192
BC = BATCH // N_CORES          # 1024 batch rows per core
NB = BC // 128                 # 8 partition tiles per core
IN_DIM = 784
HID = 128
CLS = 10
ENSEMBLE = 16
INTERNAL = 255                 # internal nodes per tree
TOTAL = 511                    # all nodes per tree
DEPTH = 9
NODES = ENSEMBLE * INTERNAL    # 4080 decision columns
NCHUNK = 510                   # nodes per matmul (<=512 fp32 psum bank)
NCHUNKS = NODES // NCHUNK      # 8
K_SIZES = [128] * 6 + [16]     # 784 = 6*128 + 16 contraction chunks


def _split_bf16(a):
    """Exact split a = hi + lo with hi, lo bf16 (lo catches the tail)."""
    hi = a.astype(ml_dtypes.bfloat16)
    lo = (a - hi.astype(np.float32)).astype(ml_dtypes.bfloat16)
    return np.ascontiguousarray(hi), np.ascontiguousarray(lo)


def build_bass(with_nodeb, with_b2):
    nc = bacc.Bacc("TRN2", target_bir_lowering=False, debug=False,
                   num_devices=N_CORES)

    xth_d = nc.dram_tensor("xth", [IN_DIM, BC], BF16, kind="ExternalInput")
    xtl_d = nc.dram_tensor("xtl", [IN_DIM, BC], BF16, kind="ExternalInput")
    w1h_d = nc.dram_tensor("w1h", [IN_DIM, HID], BF16, kind="ExternalInput")
    w1l_d = nc.dram_tensor("w1l", [IN_DIM, HID], BF16, kind="ExternalInput")
    b1_d = nc.dram_tensor("b1", [HID, 1], F32, kind="ExternalInput")
    w2_d = nc.dram_tensor("w2", [HID, CLS], F32, kind="ExternalInput")
    nwh_d = nc.dram_tensor("nwh", [HID, NODES], BF16, kind="ExternalInput")
    nwl_d = nc.dram_tensor("nwl", [HID, NODES], BF16, kind="ExternalInput")
    b2_d = (nc.dram_tensor("b2", [1, CLS], F32, kind="ExternalInput")
            if with_b2 else None)
    nb_d = (nc.dram_tensor("nb", [1, NODES], F32, kind="ExternalInput")
            if with_nodeb else None)

    pred_d = nc.dram_tensor("pred", [BC, CLS], F32, kind="ExternalOutput")
    res_d = nc.dram_tensor("res", [NB, 128, ENSEMBLE, TOTAL], F32,
                           kind="ExternalOutput")

    with tile.TileContext(nc) as tc, ExitStack() as ctx:
        consts = ctx.enter_context(tc.tile_pool(name="consts", bufs=1))
        xtp = ctx.enter_context(tc.tile_pool(name="xtp", bufs=1))
        fp = ctx.enter_context(tc.tile_pool(name="fp", bufs=1))
        enc_ps = ctx.enter_context(
            tc.tile_pool(name="enc_ps", bufs=2, space="PSUM"))
        pred_ps = ctx.enter_context(
            tc.tile_pool(name="pred_ps", bufs=2, space="PSUM"))
        tree_ps = ctx.enter_context(
            tc.tile_pool(name="tree_ps", bufs=2, space="PSUM"))
        decp = ctx.enter_context(tc.tile_pool(name="decp", bufs=2))
        resp = ctx.enter_context(tc.tile_pool(name="resp", bufs=2))
        outp = ctx.enter_context(tc.tile_pool(name="outp", bufs=2))

        # ---- constant/weight loads ----
        nwh_sb = consts.tile([HID, NODES], BF16, tag="nwh")
        nwl_sb = consts.tile([HID, NODES], BF16, tag="nwl")
        nc.sync.dma_start(out=nwh_sb, in_=nwh_d[:])
        nc.sync.dma_start(out=nwl_sb, in_=nwl_d[:])
        w1h_sb, w1l_sb = [], []
        off = 0
        for k, ksz in enumerate(K_SIZES):
            th = consts.tile([128, HID], BF16, tag=f"w1h_{k}")
            tl = consts.tile([128, HID], BF16, tag=f"w1l_{k}")
            nc.sync.dma_start(out=th[:ksz], in_=w1h_d[off:off + ksz, :])
            nc.sync.dma_start(out=tl[:ksz], in_=w1l_d[off:off + ksz, :])
            w1h_sb.append(th)
            w1l_sb.append(tl)
            off += ksz
        b1_sb = consts.tile([HID, 1], F32, tag="b1")
        nc.sync.dma_start(out=b1_sb, in_=b1_d[:])
        w2_sb = consts.tile([HID, CLS], F32, tag="w2")
        nc.sync.dma_start(out=w2_sb, in_=w2_d[:])
        ones_sb = consts.tile([1, 128], F32, tag="ones")
        nc.vector.memset(ones_sb, 1.0)
        if with_b2:
            b2_sb = consts.tile([1, CLS], F32, tag="b2")
            nc.sync.dma_start(out=b2_sb, in_=b2_d[:])
        if with_nodeb:
            nb_sb = consts.tile([1, NODES], F32, tag="nb")
            nc.sync.dma_start(out=nb_sb, in_=nb_d[:])

        xth_sb, xtl_sb = [], []
        off = 0
        for k, ksz in enumerate(K_SIZES):
            th = xtp.tile([128, BC], BF16, tag=f"xth_{k}")
            tl = xtp.tile([128, BC], BF16, tag=f"xtl_{k}")
            nc.sync.dma_start(out=th[:ksz], in_=xth_d[off:off + ksz, :])
            nc.sync.dma_start(out=tl[:ksz], in_=xtl_d[off:off + ksz, :])
            xth_sb.append(th)
            xtl_sb.append(tl)
            off += ksz

        # ---- encoder: featsT[h, b] = relu(X @ W1 + b1)^T, 3-pass bf16 ----
        featsT = fp.tile([HID, BC], F32, tag="featsT")
        nk = len(K_SIZES)
        for n in range(BC // 512):
            ps = enc_ps.tile([128, 512], F32, tag="enc")
            nsl = slice(n * 512, (n + 1) * 512)
            mms = []
            for k, ksz in enumerate(K_SIZES):
                mms += [(w1h_sb[k], xth_sb[k], ksz),
                        (w1h_sb[k], xtl_sb[k], ksz),
                        (w1l_sb[k], xth_sb[k], ksz)]
            for j, (wt, xt, ksz) in enumerate(mms):
                nc.tensor.matmul(ps, wt[:ksz], xt[:ksz, nsl],
                                 start=(j == 0), stop=(j == len(mms) - 1))
            nc.scalar.activation(out=featsT[:, nsl], in_=ps,
                                 func=AF.Relu, bias=b1_sb, scale=1.0)

        # device-side exact split featsT = fhi + flo (bf16 each)
        fhi = fp.tile([HID, BC], BF16, tag="fhi")
        flo = fp.tile([HID, BC], BF16, tag="flo")
        fdiff = fp.tile([HID, BC], F32, tag="fdiff")
        nc.vector.tensor_copy(fhi, featsT)
        nc.vector.tensor_sub(fdiff, featsT, fhi)
        nc.vector.tensor_copy(flo, fdiff)

        # ---- aux prediction: pred[b, c] = feats @ W2 + b2 ----
        for i in range(NB):
            bsl = featsT[:, i * 128:(i + 1) * 128]
            pp = pred_ps.tile([128, CLS], F32, tag="pred")
            nc.tensor.matmul(pp, bsl, w2_sb, start=True, stop=not with_b2)
            if with_b2:
                nc.tensor.matmul(pp, ones_sb, b2_sb, start=False, stop=True)
            po = outp.tile([128, CLS], F32, tag="po")
            nc.vector.tensor_copy(po, pp)
            nc.sync.dma_start(out=pred_d[i * 128:(i + 1) * 128, :], in_=po)

        # ---- trees, per 128-row batch tile ----
        for i in range(NB):
            isl = slice(i * 128, (i + 1) * 128)
            dec = decp.tile([128, NODES], F32, tag="dec")
            for half in range(NCHUNKS // 2):
                ps = tree_ps.tile([128, 2, 512], F32, tag="tree")
                for c2 in range(2):
                    c = half * 2 + c2
                    csl = slice(c * NCHUNK, (c + 1) * NCHUNK)
                    out_ps = ps[:, c2, :NCHUNK]
                    nc.tensor.matmul(out_ps, fhi[:, isl], nwh_sb[:, csl],
                                     start=True, stop=False)
                    nc.tensor.matmul(out_ps, flo[:, isl], nwh_sb[:, csl],
                                     start=False, stop=False)
                    nc.tensor.matmul(out_ps, fhi[:, isl], nwl_sb[:, csl],
                                     start=False, stop=not with_nodeb)
                    if with_nodeb:
                        nc.tensor.matmul(out_ps, ones_sb, nb_sb[:, csl],
                                         start=False, stop=True)
                dsl = dec[:, half * 2 * NCHUNK:(half + 1) * 2 * NCHUNK]
                nc.scalar.activation(
                    out=dsl.rearrange("p (x i) -> p x i", x=2),
                    in_=ps[:, :, :NCHUNK], func=AF.Sigmoid)

            dec16 = dec.rearrange("p (t i) -> p t i", t=ENSEMBLE)
            res_sb = resp.tile([128, ENSEMBLE, TOTAL], F32, tag="res")
            nc.vector.memset(res_sb[:, :, 0:1], 1.0)
            for lvl in range(DEPTH - 1):
                start = (1 << lvl) - 1
                cnt = 1 << lvl
                parent = res_sb[:, :, start:start + cnt]
                decs = dec16[:, :, start:start + cnt]
                ch = res_sb[:, :, 2 * start + 1: 2 * start + 1 + 2 * cnt]
                ch = ch.rearrange("p t (k two) -> p t k two", two=2)
                left = ch[:, :, :, 0]
                right = ch[:, :, :, 1]
                nc.vector.tensor_mul(left, parent, decs)
                nc.vector.tensor_sub(right, parent, left)
            nc.sync.dma_start(out=res_d[i], in_=res_sb)

    nc.compile()
    return nc


_NC_CACHE = {}


def _get_nc(with_nodeb, with_b2):
    key = (with_nodeb, with_b2)
    if key not in _NC_CACHE:
        _NC_CACHE[key] = build_bass(with_nodeb, with_b2)
    return _NC_CACHE[key]


def run_on_device(inputs, trace=False):
    """Shard inputs, run the Bass kernel on 8 cores, gather full outputs.

    Returns (prediction, all_results, BassKernelResults)."""
    X = np.ascontiguousarray(np.asarray(inputs["X"], dtype=np.float32))
    W1 = np.ascontiguousarray(np.asarray(inputs["W1"], dtype=np.float32))
    b1 = np.ascontiguousarray(
        np.asarray(inputs["b1"], dtype=np.float32).reshape(HID, 1))
    W2 = np.ascontiguousarray(np.asarray(inputs["W2"], dtype=np.float32))
    b2 = np.ascontiguousarray(
        np.asarray(inputs["b2"], dtype=np.float32).reshape(1, CLS))
    nodeW = np.asarray(inputs["nodeW"], dtype=np.float32)
    nodeb = np.ascontiguousarray(
        np.asarray(inputs["nodeb"], dtype=np.float32).reshape(1, NODES))

    XT = np.ascontiguousarray(X.T)                           # [784, 8192]
    xth, xtl = _split_bf16(XT)
    w1h, w1l = _split_bf16(W1)
    nwt = np.ascontiguousarray(nodeW.reshape(NODES, HID).T)  # [128, 4080]
    nwh, nwl = _split_bf16(nwt)

    with_nodeb = bool(np.any(nodeb))
    with_b2 = bool(np.any(b2))

    in_maps = []
    for c in range(N_CORES):
        m = {
            "xth": np.ascontiguousarray(xth[:, c * BC:(c + 1) * BC]),
            "xtl": np.ascontiguousarray(xtl[:, c * BC:(c + 1) * BC]),
            "w1h": w1h, "w1l": w1l, "b1": b1, "w2": W2,
            "nwh": nwh, "nwl": nwl,
        }
        if with_b2:
            m["b2"] = b2
        if with_nodeb:
            m["nb"] = nodeb
        in_maps.append(m)

    nc = _get_nc(with_nodeb, with_b2)
    r = run_bass_kernel_spmd(nc, in_maps, core_ids=list(range(N_CORES)),
                             trace=trace)

    prediction = np.empty((BATCH, CLS), np.float32)
    all_results = np.empty((ENSEMBLE, TOTAL, BATCH), np.float32)
    for c in range(N_CORES):
        prediction[c * BC:(c + 1) * BC] = r.results[c]["pred"]
        # res: [NB, 128, 16, 511] -> [16, 511, BC]
        rc = r.results[c]["res"].reshape(BC, ENSEMBLE, TOTAL)
        all_results[:, :, c * BC:(c + 1) * BC] = rc.transpose(1, 2, 0)
    return prediction, all_results, r


def kernel(**inputs):
    prediction, all_results, _ = run_on_device(inputs, trace=False)
    return prediction, all_results
